# revision 28
# baseline (speedup 1.0000x reference)
"""Trainium2 Bass kernel for sliding-window unfold (im2col).

reference:  out = x[:, idx, :]  with idx[w, f] = w + f
  x:   [128, 4096, 4]  f32
  out: [128, 4065, 32, 4]  f32

out[b, w] (= 128 floats = 512 B) is the contiguous slice
x[b].flat[4w : 4w + 128]; the problem is a sliding-window byte
replication and HBM write bandwidth is the roofline.  Per core
(16 batches): 33.3 MB of output writes through 16 SDMA engines capped
at ~26.85 GB/s each, shared by loads and stores.

WPP=62 layout: each bulk tile holds TWO batches (64 partitions each,
62 windows per partition), which cuts the overlapped x-load redundancy
from 2.0x (248 f32 loaded per 124 unique) to 1.5x (372 per 248):
loads drop 2.03 MB -> 1.52 MB, all of which rides the same engine pipe
as the stores.  Stores are issued as two 31-window column pieces per
tile so every store keeps the proven shape: 128 descriptors x 15872 B,
descriptor count ~ 0 mod 16 (uniform engine spray regardless of ring
cursor).  Tile 0's first piece is split 8+23 windows so the first
store still triggers ~12 us.  Tail (windows 3968..4064) unchanged.

Device-state warning (measured 2026-08-10): exec is BIMODAL run to
run.  ~50% of runs are clean; ~25% have SDMA engine E79 degraded to
~21.5 GB/s (+10-17 us); ~25% have a ~10% uniform slowdown on ALL
engines.  The degradation is NOT caused by this kernel, persists
across runs, accumulates until the device goes
NRT_EXEC_UNIT_UNRECOVERABLE, and a device reset clears it (an earlier
session's 111 us "baseline" and its E79 doctrine were measured
entirely on a degraded device).

SWDGE descriptor->engine model (verified by HW probe runs):
  - Each dma_start's descriptors are dealt to the 16 SDMA engines in
    contiguous chunks of ceil(N/16), starting at a global ring cursor
    g that advances with every store: by N_data + one 4B
    completion-sem descriptor per participating engine for
    SBUF-sourced stores, by N_data alone for DRAM->DRAM stores.
    All stores here have N=128 (+16 sems) so spray is uniform.
  - N=120 at g==0 skips one engine (chunk 8, lanes 0..14); ragged
    counts (N % chunk != 0, e.g. 113) hit a broken ucode path (all
    descs on one engine) - keep N % 16 == 0.
  - The Tile scheduler reorders same-queue DMAs by dependency
    readiness; add_dep_helper(sync=False) edges are NOT honored, so
    ring-cursor-sensitive store sequences cannot be pinned cheaply.
    Skewing bytes away from E79 (120+8 pairs) works when aligned but
    costs +1.5-2 us clean and only pays in ~25% of runs -> rejected.
  - Descriptors <= ~768 B aggregate into multi-desc packets; D2D
    stores with strided src run at ~10-13 GB/s/engine.  A zero-dep
    D2D tail store starves the HW-queue loads (+5 us) -> rejected.
  - Tile inserts WAW semaphores between DMAs with overlapping DRAM
    ranges - keep all output writes strictly disjoint.
  - Every dma_start costs ~0.6 us trigger time; completion semaphores
    fire ~2 us after the last byte.
  - Also tried, neutral-to-negative on clean runs: SWDGE warmup
    store, head-slice X0a load, store reorders, load-ring rebalance.
    The drain start is gated by load traffic occupying the engines
    until ~15 us, not by the first store's descriptor chain.
"""

import numpy as np

from concourse import bacc, mybir, tile
from concourse.bass_utils import run_bass_kernel_spmd

N_CORES = 8
B_FULL = 128
B = B_FULL // N_CORES  # 16 batches per core
S = 4096
C = 4
F = 32
W = S - F + 1    # 4065
FL = F * C       # 128 floats per window
XB = S * C       # 16384 floats per batch of x
OB = W * FL      # 520320 floats per batch of out

WPP = 62                   # windows per partition (2 batches per tile)
HB = 64                    # partitions per batch within a tile
NT = B // 2                # 8 bulk tiles
YROW = WPP * FL            # 7936 floats per partition row
XROW = (WPP - 1) * C + FL  # 372 floats of x per partition
PC = 31                    # windows per store/expand piece
PCE = PC * FL              # 3968 floats per piece per partition
NBULK = HB * WPP           # 3968 bulk windows per batch (as before)

# tail geometry: windows 3968..4063 as 8 slices of 12 windows per batch
# (partition p = 8*b + s, strictly disjoint writes); window 4064 is a
# [16, 128] raw load+store (partition = batch, contiguous 512 B rows).
TSL = 8                    # slices per batch
TWIN = 12                  # windows per slice
TSTR = 12                  # window stride between slices
TROW = TWIN * FL           # 1536 floats of tail output per partition
RLD = 176                  # floats of raw x loaded per partition
W4 = W - 1                 # window 4064
H0 = 8                     # windows in tile-0 first piece (small so the
                           # first store triggers ~12 us)
H1 = PC - H0               # 23 windows in the second piece (ACT)

_cache = {}


def build_nc():
    nc = bacc.Bacc("TRN2", target_bir_lowering=False)
    x = nc.dram_tensor("x", [B, S, C], mybir.dt.float32, kind="ExternalInput")
    out = nc.dram_tensor("out", [B, W, F, C], mybir.dt.float32, kind="ExternalOutput")

    with tile.TileContext(nc) as tc:
        with (
            tc.tile_pool(name="xp", bufs=1) as xp,
            tc.tile_pool(name="yp", bufs=10) as yp,
            tc.tile_pool(name="y0a", bufs=1) as y0ap,
            tc.tile_pool(name="y0b", bufs=1) as y0bp,
            tc.tile_pool(name="rp", bufs=1) as rp,
            tc.tile_pool(name="vp", bufs=1) as vp,
            tc.tile_pool(name="tp", bufs=1) as tp,
        ):
            def ld(engine, dst_tile, dst_ap, dst_off, src_ap, src_off):
                src = x[:].copy()
                src.ap = mybir.VecI64Pair(src_ap)
                src.offset = src_off
                dst = dst_tile[:].copy()
                dst.ap = mybir.VecI64Pair(dst_ap)
                dst.offset = dst_off
                engine.dma_start(out=dst, in_=src)

            def st(engine, src_tile, src_ap, src_off, dst_ap, dst_off):
                dst = out[:].copy()
                dst.ap = mybir.VecI64Pair(dst_ap)
                dst.offset = dst_off
                src = src_tile[:].copy()
                src.ap = mybir.VecI64Pair(src_ap)
                src.offset = src_off
                return engine.dma_start(out=dst, in_=src)

            def expandp(engine, xa, t, yt, yrow, j0, nwin):
                # windows [j0, j0+nwin) of tile t's 62-window rows into a
                # dedicated piece tile with row length yrow
                src = xa[:].copy()
                src.ap = mybir.VecI64Pair(
                    [[NT * XROW, 128], [C, nwin], [1, FL]])
                src.offset = t * XROW + j0 * C
                dst = yt[:].copy()
                dst.ap = mybir.VecI64Pair([[yrow, 128], [FL, nwin], [1, FL]])
                dst.offset = 0
                if engine is nc.vector:
                    engine.tensor_copy(out=dst, in_=src)
                else:
                    engine.copy(out=dst, in_=src)

            def stp(yt, yrow, t, j0, nwin):
                # store a full piece tile (row = yrow = nwin*FL, flat-
                # mergeable src) into windows [j0, j0+nwin) of both
                # batches of tile t: 128 descriptors of nwin*FL f32,
                # chunk 8 -> 8 consecutive rows per engine.
                return st(nc.gpsimd, yt,
                          [[yrow, 128], [1, nwin * FL]], 0,
                          [[OB, 2], [YROW, HB], [1, nwin * FL]],
                          2 * t * OB + j0 * FL)

            # ---- loads ----
            # ONE X tile [128, 8*372]: partition 64i+q, column group t
            # holds x[2t+i].flat[248q : 248q+372].  4 load instructions
            # total (2 per HWDGE ring) - a HW queue blocks its engine's
            # instruction FIFO when more than ~4 DMAs are outstanding,
            # which is what delayed the ACT expands to ~29us with
            # per-tile loads (measured).  Tile 0's halves are separate
            # so the first expand's semaphore arrives ~12us.
            XA = xp.tile([128, NT * XROW], mybir.dt.float32)
            XROWS = NT * XROW
            ld(nc.sync, XA, [[XROWS, HB], [1, XROW]], 0,
               [[WPP * C, HB], [1, XROW]], 0)
            ld(nc.sync, XA, [[XROWS, HB], [XROW, NT - 1], [1, XROW]], XROW,
               [[WPP * C, HB], [2 * XB, NT - 1], [1, XROW]], 2 * XB)
            V = vp.tile([16, FL], mybir.dt.float32)
            ld(nc.scalar, V, [[FL, 16], [1, FL]], 0,
               [[XB, B], [1, FL]], W4 * C)
            R = rp.tile([128, RLD], mybir.dt.float32)
            ld(nc.scalar, R, [[RLD, 128], [1, RLD]], 0,
               [[XB, B], [TSTR * C, TSL], [1, RLD]], NBULK * C)
            ld(nc.scalar, XA, [[XROWS, HB], [1, XROW]], HB * XROWS,
               [[WPP * C, HB], [1, XROW]], XB)
            ld(nc.scalar, XA, [[XROWS, HB], [XROW, NT - 1], [1, XROW]],
               HB * XROWS + XROW,
               [[WPP * C, HB], [2 * XB, NT - 1], [1, XROW]], 3 * XB)

            # ---- expands ----
            # One [128, 3968] tile per 31-window piece: full rows merge
            # flat on the SBUF side, which is the REQUIRED shape for a
            # 3-dim (2-batch) DRAM dst - a structured src against a
            # 3-dim dst makes the ucode emit 4B descriptors (measured),
            # and 31744B descriptors run at half rate (measured).  This
            # keeps the baseline's exact store economics: 128 descs x
            # 15872 B, chunk 8, 127 KB contiguous per engine per store.
            # DVE: tile-0 head piece, then every tile's second piece.
            # ACT: tile-0 second head piece, tail expand, then tiles
            # 1..7 first pieces.  ~35 us per engine, well under drain.
            Ys = [[yp.tile([128, PCE], mybir.dt.float32, name="Yt")
                   for j in range(2)] for t in range(NT)]
            Y0a = y0ap.tile([128, H0 * FL], mybir.dt.float32)
            expandp(nc.vector, XA, 0, Y0a, H0 * FL, 0, H0)
            Y0b = y0bp.tile([128, H1 * FL], mybir.dt.float32)
            expandp(nc.scalar, XA, 0, Y0b, H1 * FL, H0, H1)
            T = tp.tile([128, TROW], mybir.dt.float32)
            tsrc = R[:].copy()
            tsrc.ap = mybir.VecI64Pair([[RLD, 128], [C, TWIN], [1, FL]])
            tsrc.offset = 0
            tdst = T[:].copy()
            tdst.ap = mybir.VecI64Pair([[TROW, 128], [FL, TWIN], [1, FL]])
            tdst.offset = 0
            nc.scalar.copy(out=tdst, in_=tsrc)
            for t in range(NT):
                expandp(nc.vector, XA, t, Ys[t][1], PCE, PC, PC)
                if t >= 1:
                    expandp(nc.scalar, XA, t, Ys[t][0], PCE, 0, PC)

            # ---- stores: ALL on GPSIMD/SWDGE, FIFO order by earliest
            # dependency.  Every store: 128 descriptors, disjoint dst.
            st(nc.gpsimd, V, [[FL, 16], [1, FL]], 0,
               [[OB, B], [1, FL]], W4 * FL)
            stp(Y0a, H0 * FL, 0, 0, H0)
            stp(Y0b, H1 * FL, 0, H0, H1)
            st(nc.gpsimd, T, [[TROW, 128], [1, TROW]], 0,
               [[OB, B], [TSTR * FL, TSL], [1, TROW]], NBULK * FL)
            stp(Ys[0][1], PCE, 0, PC, PC)
            for t in range(1, NT):
                stp(Ys[t][0], PCE, t, 0, PC)
                stp(Ys[t][1], PCE, t, PC, PC)

    nc.finalize()
    return nc


def run_sharded(x: np.ndarray, trace: bool = False):
    """Shard batch across 8 cores, run, gather. Returns (out, raw results)."""
    if "nc" not in _cache:
        _cache["nc"] = build_nc()
    nc = _cache["nc"]

    x = np.ascontiguousarray(x, dtype=np.float32)
    in_maps = [{"x": x[i * B : (i + 1) * B]} for i in range(N_CORES)]
    res = run_bass_kernel_spmd(nc, in_maps, list(range(N_CORES)), trace=trace)
    out = np.concatenate([res.results[i]["out"] for i in range(N_CORES)], axis=0)
    return out, res


def kernel(x: np.ndarray) -> np.ndarray:
    out, _ = run_sharded(x, trace=False)
    return out


# revision 29
# speedup vs baseline: 1.3441x; 1.3441x over previous
"""Trainium2 Bass kernel for sliding-window unfold (im2col).

reference:  out = x[:, idx, :]  with idx[w, f] = w + f
  x:   [128, 4096, 4]  f32
  out: [128, 4065, 32, 4]  f32

out[b, w] (= 128 floats = 512 B) is the contiguous slice
x[b].flat[4w : 4w + 128]; the problem is a sliding-window byte
replication and HBM write bandwidth is the roofline.  Per core
(16 batches): 33.3 MB of output writes through 16 SDMA engines capped
at ~26.85 GB/s each, shared by loads and stores.

WPP=62 layout: each bulk tile holds TWO batches (64 partitions each,
62 windows per partition), which cuts the overlapped x-load redundancy
from 2.0x (248 f32 loaded per 124 unique) to 1.5x (372 per 248):
loads drop 2.03 MB -> 1.52 MB, all of which rides the same engine pipe
as the stores.  Stores are issued as two 31-window column pieces per
tile so every store keeps the proven shape: 128 descriptors x 15872 B,
descriptor count ~ 0 mod 16 (uniform engine spray regardless of ring
cursor).  Tile 0's first piece is split 8+23 windows so the first
store still triggers ~12 us.  Tail (windows 3968..4064) unchanged.

Device-state warning (measured 2026-08-10): exec is BIMODAL run to
run.  ~50% of runs are clean; ~25% have SDMA engine E79 degraded to
~21.5 GB/s (+10-17 us); ~25% have a ~10% uniform slowdown on ALL
engines.  The degradation is NOT caused by this kernel, persists
across runs, accumulates until the device goes
NRT_EXEC_UNIT_UNRECOVERABLE, and a device reset clears it (an earlier
session's 111 us "baseline" and its E79 doctrine were measured
entirely on a degraded device).

SWDGE descriptor->engine model (verified by HW probe runs):
  - Each dma_start's descriptors are dealt to the 16 SDMA engines in
    contiguous chunks of ceil(N/16), starting at a global ring cursor
    g that advances with every store: by N_data + one 4B
    completion-sem descriptor per participating engine for
    SBUF-sourced stores, by N_data alone for DRAM->DRAM stores.
    All stores here have N=128 (+16 sems) so spray is uniform.
  - N=120 at g==0 skips one engine (chunk 8, lanes 0..14); ragged
    counts (N % chunk != 0, e.g. 113) hit a broken ucode path (all
    descs on one engine) - keep N % 16 == 0.
  - The Tile scheduler reorders same-queue DMAs by dependency
    readiness; add_dep_helper(sync=False) edges are NOT honored, so
    ring-cursor-sensitive store sequences cannot be pinned cheaply.
    Skewing bytes away from E79 (120+8 pairs) works when aligned but
    costs +1.5-2 us clean and only pays in ~25% of runs -> rejected.
  - Descriptors <= ~768 B aggregate into multi-desc packets; D2D
    stores with strided src run at ~10-13 GB/s/engine.  A zero-dep
    D2D tail store starves the HW-queue loads (+5 us) -> rejected.
  - Tile inserts WAW semaphores between DMAs with overlapping DRAM
    ranges - keep all output writes strictly disjoint.
  - Every dma_start costs ~0.6 us trigger time; completion semaphores
    fire ~2 us after the last byte.
  - Also tried, neutral-to-negative on clean runs: SWDGE warmup
    store, head-slice X0a load, store reorders, load-ring rebalance.
    The drain start is gated by load traffic occupying the engines
    until ~15 us, not by the first store's descriptor chain.
"""

import numpy as np

from concourse import bacc, mybir, tile
from concourse.bass_utils import run_bass_kernel_spmd

N_CORES = 8
B_FULL = 128
B = B_FULL // N_CORES  # 16 batches per core
S = 4096
C = 4
F = 32
W = S - F + 1    # 4065
FL = F * C       # 128 floats per window
XB = S * C       # 16384 floats per batch of x
OB = W * FL      # 520320 floats per batch of out

WPP = 62                   # windows per partition (2 batches per tile)
HB = 64                    # partitions per batch within a tile
NT = B // 2                # 8 bulk tiles
YROW = WPP * FL            # 7936 floats per partition row
XROW = (WPP - 1) * C + FL  # 372 floats of x per partition
PC = 31                    # windows per store/expand piece
PCE = PC * FL              # 3968 floats per piece per partition
NBULK = HB * WPP           # 3968 bulk windows per batch (as before)

# tail geometry: windows 3968..4063 as 8 slices of 12 windows per batch
# (partition p = 8*b + s, strictly disjoint writes); window 4064 is a
# [16, 128] raw load+store (partition = batch, contiguous 512 B rows).
TSL = 8                    # slices per batch
TWIN = 12                  # windows per slice
TSTR = 12                  # window stride between slices
TROW = TWIN * FL           # 1536 floats of tail output per partition
RLD = 176                  # floats of raw x loaded per partition
W4 = W - 1                 # window 4064
H0 = 8                     # windows in tile-0 first piece (small so the
                           # first store triggers ~12 us)
H1 = PC - H0               # 23 windows in the second piece (ACT)

_cache = {}


def build_nc():
    nc = bacc.Bacc("TRN2", target_bir_lowering=False)
    x = nc.dram_tensor("x", [B, S, C], mybir.dt.float32, kind="ExternalInput")
    out = nc.dram_tensor("out", [B, W, F, C], mybir.dt.float32, kind="ExternalOutput")

    with tile.TileContext(nc) as tc:
        with (
            tc.tile_pool(name="xp", bufs=1) as xp,
            tc.tile_pool(name="yp", bufs=10) as yp,
            tc.tile_pool(name="y0a", bufs=1) as y0ap,
            tc.tile_pool(name="y0b", bufs=1) as y0bp,
            tc.tile_pool(name="rp", bufs=1) as rp,
            tc.tile_pool(name="vp", bufs=1) as vp,
            tc.tile_pool(name="tp", bufs=1) as tp,
        ):
            def ld(engine, dst_tile, dst_ap, dst_off, src_ap, src_off):
                src = x[:].copy()
                src.ap = mybir.VecI64Pair(src_ap)
                src.offset = src_off
                dst = dst_tile[:].copy()
                dst.ap = mybir.VecI64Pair(dst_ap)
                dst.offset = dst_off
                engine.dma_start(out=dst, in_=src)

            def st(engine, src_tile, src_ap, src_off, dst_ap, dst_off):
                dst = out[:].copy()
                dst.ap = mybir.VecI64Pair(dst_ap)
                dst.offset = dst_off
                src = src_tile[:].copy()
                src.ap = mybir.VecI64Pair(src_ap)
                src.offset = src_off
                return engine.dma_start(out=dst, in_=src)

            def expandp(engine, xa, t, yt, yrow, j0, nwin):
                # windows [j0, j0+nwin) of tile t's 62-window rows into a
                # dedicated piece tile with row length yrow
                src = xa[:].copy()
                src.ap = mybir.VecI64Pair(
                    [[NT * XROW, 128], [C, nwin], [1, FL]])
                src.offset = t * XROW + j0 * C
                dst = yt[:].copy()
                dst.ap = mybir.VecI64Pair([[yrow, 128], [FL, nwin], [1, FL]])
                dst.offset = 0
                if engine is nc.vector:
                    engine.tensor_copy(out=dst, in_=src)
                else:
                    engine.copy(out=dst, in_=src)

            def stp(yt, yrow, t, i, j0, nwin):
                # store batch-half i of a piece tile into windows
                # [j0, j0+nwin) of batch 2t+i: 64 descriptors of
                # nwin*FL f32, flat-mergeable src, 2-dim dst.  A true
                # 3-dim dst (batch jump + partition stride + run) makes
                # the ucode emit ~1k extra 4B ring descriptors per store
                # and scrambles the engine spray (measured twice).
                return st(nc.gpsimd, yt,
                          [[yrow, HB], [1, yrow]], i * HB * yrow,
                          [[YROW, HB], [1, nwin * FL]],
                          (2 * t + i) * OB + j0 * FL)

            # ---- loads ----
            # ONE X tile [128, 8*372]: partition 64i+q, column group t
            # holds x[2t+i].flat[248q : 248q+372].  4 load instructions
            # total (2 per HWDGE ring) - a HW queue blocks its engine's
            # instruction FIFO when more than ~4 DMAs are outstanding,
            # which is what delayed the ACT expands to ~29us with
            # per-tile loads (measured).  Tile 0's halves are separate
            # so the first expand's semaphore arrives ~12us.
            XA = xp.tile([128, NT * XROW], mybir.dt.float32)
            XROWS = NT * XROW
            ld(nc.sync, XA, [[XROWS, HB], [1, XROW]], 0,
               [[WPP * C, HB], [1, XROW]], 0)
            ld(nc.sync, XA, [[XROWS, HB], [XROW, NT - 1], [1, XROW]], XROW,
               [[WPP * C, HB], [2 * XB, NT - 1], [1, XROW]], 2 * XB)
            V = vp.tile([16, FL], mybir.dt.float32)
            ld(nc.scalar, V, [[FL, 16], [1, FL]], 0,
               [[XB, B], [1, FL]], W4 * C)
            R = rp.tile([128, RLD], mybir.dt.float32)
            ld(nc.scalar, R, [[RLD, 128], [1, RLD]], 0,
               [[XB, B], [TSTR * C, TSL], [1, RLD]], NBULK * C)
            ld(nc.scalar, XA, [[XROWS, HB], [1, XROW]], HB * XROWS,
               [[WPP * C, HB], [1, XROW]], XB)
            ld(nc.scalar, XA, [[XROWS, HB], [XROW, NT - 1], [1, XROW]],
               HB * XROWS + XROW,
               [[WPP * C, HB], [2 * XB, NT - 1], [1, XROW]], 3 * XB)

            # ---- expands ----
            # One [128, 3968] tile per 31-window piece: full rows merge
            # flat on the SBUF side, which is the REQUIRED shape for a
            # 3-dim (2-batch) DRAM dst - a structured src against a
            # 3-dim dst makes the ucode emit 4B descriptors (measured),
            # and 31744B descriptors run at half rate (measured).  This
            # keeps the baseline's exact store economics: 128 descs x
            # 15872 B, chunk 8, 127 KB contiguous per engine per store.
            # DVE: tile-0 head piece, then every tile's second piece.
            # ACT: tile-0 second head piece, tail expand, then tiles
            # 1..7 first pieces.  ~35 us per engine, well under drain.
            Ys = [[yp.tile([128, PCE], mybir.dt.float32, name="Yt")
                   for j in range(2)] for t in range(NT)]
            Y0a = y0ap.tile([128, H0 * FL], mybir.dt.float32)
            expandp(nc.vector, XA, 0, Y0a, H0 * FL, 0, H0)
            Y0b = y0bp.tile([128, H1 * FL], mybir.dt.float32)
            expandp(nc.scalar, XA, 0, Y0b, H1 * FL, H0, H1)
            T = tp.tile([128, TROW], mybir.dt.float32)
            tsrc = R[:].copy()
            tsrc.ap = mybir.VecI64Pair([[RLD, 128], [C, TWIN], [1, FL]])
            tsrc.offset = 0
            tdst = T[:].copy()
            tdst.ap = mybir.VecI64Pair([[TROW, 128], [FL, TWIN], [1, FL]])
            tdst.offset = 0
            nc.scalar.copy(out=tdst, in_=tsrc)
            for t in range(NT):
                expandp(nc.vector, XA, t, Ys[t][1], PCE, PC, PC)
                if t >= 1:
                    expandp(nc.scalar, XA, t, Ys[t][0], PCE, 0, PC)

            # ---- stores: ALL on GPSIMD/SWDGE, FIFO order by earliest
            # dependency.  Every store: 128 descriptors, disjoint dst.
            st(nc.gpsimd, V, [[FL, 16], [1, FL]], 0,
               [[OB, B], [1, FL]], W4 * FL)
            stp(Y0a, H0 * FL, 0, 0, 0, H0)
            stp(Y0a, H0 * FL, 0, 1, 0, H0)
            stp(Y0b, H1 * FL, 0, 0, H0, H1)
            stp(Y0b, H1 * FL, 0, 1, H0, H1)
            st(nc.gpsimd, T, [[TROW, 128], [1, TROW]], 0,
               [[OB, B], [TSTR * FL, TSL], [1, TROW]], NBULK * FL)
            stp(Ys[0][1], PCE, 0, 0, PC, PC)
            stp(Ys[0][1], PCE, 0, 1, PC, PC)
            for t in range(1, NT):
                for i in range(2):
                    stp(Ys[t][0], PCE, t, i, 0, PC)
                    stp(Ys[t][1], PCE, t, i, PC, PC)

    nc.finalize()
    return nc


def run_sharded(x: np.ndarray, trace: bool = False):
    """Shard batch across 8 cores, run, gather. Returns (out, raw results)."""
    if "nc" not in _cache:
        _cache["nc"] = build_nc()
    nc = _cache["nc"]

    x = np.ascontiguousarray(x, dtype=np.float32)
    in_maps = [{"x": x[i * B : (i + 1) * B]} for i in range(N_CORES)]
    res = run_bass_kernel_spmd(nc, in_maps, list(range(N_CORES)), trace=trace)
    out = np.concatenate([res.results[i]["out"] for i in range(N_CORES)], axis=0)
    return out, res


def kernel(x: np.ndarray) -> np.ndarray:
    out, _ = run_sharded(x, trace=False)
    return out


# revision 30
# speedup vs baseline: 1.6925x; 1.2592x over previous
"""Trainium2 Bass kernel for sliding-window unfold (im2col).

reference:  out = x[:, idx, :]  with idx[w, f] = w + f
  x:   [128, 4096, 4]  f32
  out: [128, 4065, 32, 4]  f32

out[b, w] (= 128 floats = 512 B) is the contiguous slice
x[b].flat[4w : 4w + 128]; the problem is a sliding-window byte
replication and HBM write bandwidth is the roofline.  Per core
(16 batches): 33.3 MB of output writes.  A deep SWDGE store queue
sustains ~420-435 GB/s; floor = ~78 us of store drain + ~10 us fixed
framework pre/post-amble + ~4 us ramp.

Hard-won scheduling facts (from NTFF traces of prior iterations):
  - DGE descriptor generation is serial at ~5-10 ns/desc; descriptor
    size = the final contiguous AP dim.  512 B-chunk DMAs top out near
    ~100 GB/s; big descriptors are everything.
  - HWDGE *stores* persistently degrade SDMA engine 15 to ~21 GB/s
    (vs 26.5) while other engines are active, unbalancing the drain by
    ~18 us.  All stores go on GPSIMD/SWDGE; HWDGE carries loads only.
  - A store whose DRAM-side partition stride is NOT uniform (e.g. a
    3-dim dst mixing a batch jump with a slice stride) can break the
    descriptor spray: engines get unequal bytes and per-packet rate
    halves; worst case it degenerates to 4 B descriptors.  The proven
    store shape is dst [[row,128],[1,row]]-style with uniform stride.
  - Tile inserts WAW semaphores between DMAs with overlapping DRAM
    ranges - keep all output writes strictly disjoint.
  - A DVE copy that enters 2-port perf mode locks GPSIMD off the
    shared SBUF port and stalls SWDGE descriptor emission.
  - Every dma_start costs ~0.6 us of trigger time on its issuing
    engine; completions fire ~2 us after the last byte.

Layout (per core):
  bulk: partition p holds windows 31p..31p+30 of one batch b.
    load X (248 f32/partition/batch), expand on ACT/DVE into
    Y[128, 3968] via an overlapping-stride read AP, store Y ->
    out[b] windows 0..3967 (contiguous 15.5 KB per partition ->
    128 fat descriptors at 26.5 GB/s/engine).  Batch 0's expand is
    split in half across DVE and ACT so the first store issues ~8.5 us.
  tail: windows 3968..4063 (disjoint from bulk): partition p = 8b+s
    holds 12 consecutive windows of batch b's tail (6 KB descriptors),
    expanded from a tiny raw load.  Window 4064 is contiguous x data:
    a [16, 128] tile (partition = batch) rides a single-engine
    load+store of 512 B descriptors, issued first to prime the pipe.
"""

import numpy as np

from concourse import bacc, mybir, tile
from concourse.bass_utils import run_bass_kernel_spmd

N_CORES = 8
B_FULL = 128
B = B_FULL // N_CORES  # 16 batches per core
S = 4096
C = 4
F = 32
W = S - F + 1    # 4065
FL = F * C       # 128 floats per window
XB = S * C       # 16384 floats per batch of x
OB = W * FL      # 520320 floats per batch of out
WPP = 31         # windows per partition in the bulk store
NBULK = 128 * WPP          # 3968 bulk windows per batch
YROW = WPP * FL            # 3968 floats per partition row
XROW = (WPP - 1) * C + FL  # 248 floats of x per partition per batch

# tail geometry: windows 3968..4063 as 8 slices of 12 windows per batch
# (partition p = 8*b + s, strictly disjoint writes); window 4064 is a
# [16, 128] raw load+store (partition = batch, contiguous 512 B rows).
TSL = 8                    # slices per batch
TWIN = 12                  # windows per slice
TSTR = 12                  # window stride between slices
TROW = TWIN * FL           # 1536 floats of tail output per partition
RLD = 176                  # floats of raw x loaded per partition
W4 = W - 1                 # window 4064
H0 = 8                     # windows in bulk batch-0 first piece (small
                           # so the first store issues ~7.6 us)
H1 = WPP - H0              # 23 windows in second piece (ACT)

_cache = {}


def build_nc():
    nc = bacc.Bacc("TRN2", target_bir_lowering=False)
    x = nc.dram_tensor("x", [B, S, C], mybir.dt.float32, kind="ExternalInput")
    out = nc.dram_tensor("out", [B, W, F, C], mybir.dt.float32, kind="ExternalOutput")

    with tile.TileContext(nc) as tc:
        with (
            tc.tile_pool(name="x01", bufs=2) as x01,
            tc.tile_pool(name="xg1", bufs=1) as xg1p,
            tc.tile_pool(name="xg2", bufs=1) as xg2p,
            tc.tile_pool(name="y0a", bufs=1) as y0ap,
            tc.tile_pool(name="y0b", bufs=1) as y0bp,
            tc.tile_pool(name="yp", bufs=10) as yp,
            tc.tile_pool(name="rp", bufs=1) as rp,
            tc.tile_pool(name="vp", bufs=1) as vp,
            tc.tile_pool(name="tp", bufs=1) as tp,
        ):
            def ld(engine, dst_tile, dst_ap, dst_off, src_ap, src_off):
                src = x[:].copy()
                src.ap = mybir.VecI64Pair(src_ap)
                src.offset = src_off
                dst = dst_tile[:].copy()
                dst.ap = mybir.VecI64Pair(dst_ap)
                dst.offset = dst_off
                engine.dma_start(out=dst, in_=src)

            def st(engine, src_tile, src_ap, src_off, dst_ap, dst_off):
                dst = out[:].copy()
                dst.ap = mybir.VecI64Pair(dst_ap)
                dst.offset = dst_off
                src = src_tile[:].copy()
                src.ap = mybir.VecI64Pair(src_ap)
                src.offset = src_off
                engine.dma_start(out=dst, in_=src)

            def expand(engine, src_tile, src_row, src_off, dst_tile, dst_row, nwin):
                src = src_tile[:].copy()
                src.ap = mybir.VecI64Pair([[src_row, 128], [C, nwin], [1, FL]])
                src.offset = src_off
                dst = dst_tile[:].copy()
                dst.ap = mybir.VecI64Pair([[dst_row, 128], [FL, nwin], [1, FL]])
                dst.offset = 0
                if engine is nc.vector:
                    engine.tensor_copy(out=dst, in_=src)
                else:
                    engine.copy(out=dst, in_=src)

            # ---- loads ----
            # sync ring: batch 0, batch 1, batches 2..7
            X0 = x01.tile([128, XROW], mybir.dt.float32)
            ld(nc.sync, X0, [[XROW, 128], [1, XROW]], 0,
               [[WPP * C, 128], [1, XROW]], 0)
            X1 = x01.tile([128, XROW], mybir.dt.float32)
            ld(nc.sync, X1, [[XROW, 128], [1, XROW]], 0,
               [[WPP * C, 128], [1, XROW]], XB)
            XG1 = xg1p.tile([128, 6 * XROW], mybir.dt.float32)
            ld(nc.sync, XG1, [[6 * XROW, 128], [XROW, 6], [1, XROW]], 0,
               [[WPP * C, 128], [XB, 6], [1, XROW]], 2 * XB)
            # scalar ring: window-4064 raw load (partition = batch), tail
            # raw load, then batches 8..15
            V = vp.tile([16, FL], mybir.dt.float32)
            ld(nc.scalar, V, [[FL, 16], [1, FL]], 0,
               [[XB, B], [1, FL]], W4 * C)
            R = rp.tile([128, RLD], mybir.dt.float32)
            ld(nc.scalar, R, [[RLD, 128], [1, RLD]], 0,
               [[XB, B], [TSTR * C, TSL], [1, RLD]], NBULK * C)
            XG2 = xg2p.tile([128, 8 * XROW], mybir.dt.float32)
            ld(nc.scalar, XG2, [[8 * XROW, 128], [XROW, 8], [1, XROW]], 0,
               [[WPP * C, 128], [XB, 8], [1, XROW]], 8 * XB)

            # ---- expands ----
            # DVE: batch-0 first half, then odd batches (a 2-port-mode
            # DVE copy locks GPSIMD out of the shared SBUF port, so keep
            # DVE's queue clear while the first stores are emitted).
            Y0a = y0ap.tile([128, H0 * FL], mybir.dt.float32)
            expand(nc.vector, X0, XROW, 0, Y0a, H0 * FL, H0)
            # ACT: batch-0 second half, tail expand, then even batches
            Y0b = y0bp.tile([128, H1 * FL], mybir.dt.float32)
            expand(nc.scalar, X0, XROW, H0 * C, Y0b, H1 * FL, H1)
            T = tp.tile([128, TROW], mybir.dt.float32)
            expand(nc.scalar, R, RLD, 0, T, TROW, TWIN)

            Ys = {}
            for b in range(1, B):
                eng = nc.vector if b % 2 == 1 else nc.scalar
                if b == 1:
                    src_t, row, off = X1, XROW, 0
                elif b < 8:
                    src_t, row, off = XG1, 6 * XROW, (b - 2) * XROW
                else:
                    src_t, row, off = XG2, 8 * XROW, (b - 8) * XROW
                Y = yp.tile([128, YROW], mybir.dt.float32)
                expand(eng, src_t, row, off, Y, YROW, WPP)
                Ys[b] = Y

            # ---- stores: ALL on GPSIMD/SWDGE ----
            # window-4064 first (no expand dependency, primes the pipe;
            # 16 partitions -> one engine, 64 KB, harmless early).
            st(nc.gpsimd, V, [[FL, 16], [1, FL]], 0,
               [[OB, B], [1, FL]], W4 * FL)
            # batch-0 halves, then the tail, then batches 1..15; the
            # queue ends on clean uniform 15.5 KB-descriptor stores.
            st(nc.gpsimd, Y0a, [[H0 * FL, 128], [1, H0 * FL]], 0,
               [[YROW, 128], [1, H0 * FL]], 0)
            st(nc.gpsimd, Y0b, [[H1 * FL, 128], [1, H1 * FL]], 0,
               [[YROW, 128], [1, H1 * FL]], H0 * FL)
            st(nc.gpsimd, T, [[TROW, 128], [1, TROW]], 0,
               [[OB, B], [TSTR * FL, TSL], [1, TROW]], NBULK * FL)
            for b in range(1, B):
                st(nc.gpsimd, Ys[b], [[YROW, 128], [1, YROW]], 0,
                   [[YROW, 128], [1, YROW]], b * OB)

    nc.finalize()
    return nc


def run_sharded(x: np.ndarray, trace: bool = False):
    """Shard batch across 8 cores, run, gather. Returns (out, raw results)."""
    if "nc" not in _cache:
        _cache["nc"] = build_nc()
    nc = _cache["nc"]

    x = np.ascontiguousarray(x, dtype=np.float32)
    in_maps = [{"x": x[i * B : (i + 1) * B]} for i in range(N_CORES)]
    res = run_bass_kernel_spmd(nc, in_maps, list(range(N_CORES)), trace=trace)
    out = np.concatenate([res.results[i]["out"] for i in range(N_CORES)], axis=0)
    return out, res


def kernel(x: np.ndarray) -> np.ndarray:
    out, _ = run_sharded(x, trace=False)
    return out



# revision 35
# speedup vs baseline: 1.9128x; 1.1302x over previous
"""Trainium2 Bass kernel for sliding-window unfold (im2col).

reference:  out = x[:, idx, :]  with idx[w, f] = w + f
  x:   [128, 4096, 4]  f32
  out: [128, 4065, 32, 4]  f32

out[b, w] (= 128 floats = 512 B) is the contiguous slice
x[b].flat[4w : 4w + 128]; the problem is a sliding-window byte
replication and HBM write bandwidth is the roofline.  Per core
(16 batches): 33.3 MB of output writes.  On a healthy device this
kernel measures 100.3-101.2 us: ~6.8 us engine preamble, first load
bytes at 8.65 us, all 16 SDMA engines ~fully busy (~81 us each at the
~26.85 GB/s/engine cap; loads+stores = 35.4 MB share the same pipe)
until ~97 us, ~3.5 us completion/teardown.  This is the roofline.

Device-state warning (measured 2026-08-10): exec is BIMODAL run to
run.  ~50% of runs are clean (~100.5 us); ~25% have SDMA engine E79
degraded to ~21.5 GB/s (exec 110-118, E79 busy ~97 us vs ~81);
~25% show a ~10% uniform slowdown on all engines (105-113).  The
degradation is NOT caused by this kernel, persists across runs,
accumulates until NRT_EXEC_UNIT_UNRECOVERABLE, and a device reset
clears it (an earlier session's 111 us "baseline" and its E79
doctrine were measured entirely on a degraded device).

SWDGE laws (each verified by dedicated HW probe runs):
  - Descriptors are dealt to the 16 engines in contiguous chunks of
    ceil(N/16), from a global ring cursor g advancing by N + one 4B
    sem-desc per participating engine (SBUF src; D2D adds N only).
    N=120 at g==0 skips one engine; ragged N (N % chunk != 0, e.g.
    113) hits a broken path (all descs on one engine).
  - A store dst must reduce to <= 2 effective dims after merging
    (merged = stride equals inner run, like the tail store's
    [[OB,16],[1536,8],[1,1536]]).  A TRUE 3-dim dst emits ~1k extra
    4B ring descriptors per store and scrambles the spray; a 3-dim
    dst against a non-flat src partially degenerates to 4B descs
    (2x exec).  Proven fast: [[row,N],[1,piece]] 2-dim shapes.
  - 15872 B descriptors run at 591 ns (26.85 GB/s, the engine cap);
    4096/6144/11776 B also full rate; 31744 B run at HALF rate
    (~2255 ns) - do not exceed ~16 KB descriptors.
  - Per-engine dst contiguity/gaps do NOT matter (Y0a's 4 KB descs at
    15.9 KB stride are full rate).
  - An HWDGE queue blocks its engine's instruction FIFO when >~4 DMAs
    are outstanding - keep <= 3 load instructions per ring or the
    engine's later compute ops start 15-20 us late.
  - The Tile scheduler reorders same-queue DMAs by readiness;
    add_dep_helper(sync=False) edges are NOT honored.
  - Tile inserts WAW semaphores between DMAs with overlapping DRAM
    ranges - keep all output writes strictly disjoint.
  - Every dma_start costs ~0.6 us of trigger time on its issuing
    engine; completions fire ~2 us after the last byte.

Measured dead ends (do not retry without new evidence): E79 byte-skew
via 120+8 desc pairs (+1.5-2 us clean, pays only in the ~25% E79-mode
runs, alignment fragile); zero-dep D2D tail stores (strided-src D2D
runs 10-13 GB/s and starves loads, +5 us); SWDGE warmup store,
X0a head-slice load, store reorders, load-ring rebalance (each +1-2
us; the drain start is gated by load traffic, not descriptor
latency); WPP=62 compact-load restructure (load redundancy 2.0->1.5x
= -0.5 MB, but every store shape reachable from 2-batch tiles -
pieces, full rows, 3-dim dst - lands on one of the slow paths above;
best variant 140 us vs 100.5 here).

Layout (per core):
  bulk: partition p holds windows 31p..31p+30 of one batch b.
    load X (248 f32/partition/batch), expand on ACT/DVE into
    Y[128, 3968] via an overlapping-stride read AP, store Y ->
    out[b] windows 0..3967 (contiguous 15.5 KB per partition ->
    128 fat descriptors at 26.5 GB/s/engine).  Batch 0's expand is
    split in half across DVE and ACT so the first store issues ~8.5 us.
  tail: windows 3968..4063 (disjoint from bulk): partition p = 8b+s
    holds 12 consecutive windows of batch b's tail (6 KB descriptors),
    expanded from a tiny raw load.  Window 4064 is contiguous x data:
    a [16, 128] tile (partition = batch) rides a single-engine
    load+store of 512 B descriptors, issued first to prime the pipe.
"""

import numpy as np

from concourse import bacc, mybir, tile
from concourse.bass_utils import run_bass_kernel_spmd

N_CORES = 8
B_FULL = 128
B = B_FULL // N_CORES  # 16 batches per core
S = 4096
C = 4
F = 32
W = S - F + 1    # 4065
FL = F * C       # 128 floats per window
XB = S * C       # 16384 floats per batch of x
OB = W * FL      # 520320 floats per batch of out
WPP = 31         # windows per partition in the bulk store
NBULK = 128 * WPP          # 3968 bulk windows per batch
YROW = WPP * FL            # 3968 floats per partition row
XROW = (WPP - 1) * C + FL  # 248 floats of x per partition per batch

# tail geometry: windows 3968..4063 as 8 slices of 12 windows per batch
# (partition p = 8*b + s, strictly disjoint writes); window 4064 is a
# [16, 128] raw load+store (partition = batch, contiguous 512 B rows).
TSL = 8                    # slices per batch
TWIN = 12                  # windows per slice
TSTR = 12                  # window stride between slices
TROW = TWIN * FL           # 1536 floats of tail output per partition
RLD = 176                  # floats of raw x loaded per partition
W4 = W - 1                 # window 4064
H0 = 8                     # windows in bulk batch-0 first piece (small
                           # so the first store issues ~7.6 us)
H1 = WPP - H0              # 23 windows in second piece (ACT)

_cache = {}


def build_nc():
    nc = bacc.Bacc("TRN2", target_bir_lowering=False)
    x = nc.dram_tensor("x", [B, S, C], mybir.dt.float32, kind="ExternalInput")
    out = nc.dram_tensor("out", [B, W, F, C], mybir.dt.float32, kind="ExternalOutput")

    with tile.TileContext(nc) as tc:
        with (
            tc.tile_pool(name="x01", bufs=2) as x01,
            tc.tile_pool(name="xg1", bufs=1) as xg1p,
            tc.tile_pool(name="xg2", bufs=1) as xg2p,
            tc.tile_pool(name="y0a", bufs=1) as y0ap,
            tc.tile_pool(name="y0b", bufs=1) as y0bp,
            tc.tile_pool(name="yp", bufs=10) as yp,
            tc.tile_pool(name="rp", bufs=1) as rp,
            tc.tile_pool(name="vp", bufs=1) as vp,
            tc.tile_pool(name="tp", bufs=1) as tp,
        ):
            def ld(engine, dst_tile, dst_ap, dst_off, src_ap, src_off):
                src = x[:].copy()
                src.ap = mybir.VecI64Pair(src_ap)
                src.offset = src_off
                dst = dst_tile[:].copy()
                dst.ap = mybir.VecI64Pair(dst_ap)
                dst.offset = dst_off
                engine.dma_start(out=dst, in_=src)

            def st(engine, src_tile, src_ap, src_off, dst_ap, dst_off):
                dst = out[:].copy()
                dst.ap = mybir.VecI64Pair(dst_ap)
                dst.offset = dst_off
                src = src_tile[:].copy()
                src.ap = mybir.VecI64Pair(src_ap)
                src.offset = src_off
                engine.dma_start(out=dst, in_=src)

            def expand(engine, src_tile, src_row, src_off, dst_tile, dst_row,
                       nwin, dst_off=0):
                src = src_tile[:].copy()
                src.ap = mybir.VecI64Pair([[src_row, 128], [C, nwin], [1, FL]])
                src.offset = src_off
                dst = dst_tile[:].copy()
                dst.ap = mybir.VecI64Pair([[dst_row, 128], [FL, nwin], [1, FL]])
                dst.offset = dst_off
                if engine is nc.vector:
                    engine.tensor_copy(out=dst, in_=src)
                else:
                    engine.copy(out=dst, in_=src)

            # ---- loads ----
            # sync ring: batch 0, batch 1, batches 2..7
            X0 = x01.tile([128, XROW], mybir.dt.float32)
            ld(nc.sync, X0, [[XROW, 128], [1, XROW]], 0,
               [[WPP * C, 128], [1, XROW]], 0)
            X1 = x01.tile([128, XROW], mybir.dt.float32)
            ld(nc.sync, X1, [[XROW, 128], [1, XROW]], 0,
               [[WPP * C, 128], [1, XROW]], XB)
            XG1 = xg1p.tile([128, 6 * XROW], mybir.dt.float32)
            ld(nc.sync, XG1, [[6 * XROW, 128], [XROW, 6], [1, XROW]], 0,
               [[WPP * C, 128], [XB, 6], [1, XROW]], 2 * XB)
            # scalar ring: window-4064 raw load (partition = batch), tail
            # raw load, then batches 8..15
            V = vp.tile([16, FL], mybir.dt.float32)
            ld(nc.scalar, V, [[FL, 16], [1, FL]], 0,
               [[XB, B], [1, FL]], W4 * C)
            R = rp.tile([128, RLD], mybir.dt.float32)
            ld(nc.scalar, R, [[RLD, 128], [1, RLD]], 0,
               [[XB, B], [TSTR * C, TSL], [1, RLD]], NBULK * C)
            XG2 = xg2p.tile([128, 8 * XROW], mybir.dt.float32)
            ld(nc.scalar, XG2, [[8 * XROW, 128], [XROW, 8], [1, XROW]], 0,
               [[WPP * C, 128], [XB, 8], [1, XROW]], 8 * XB)

            # ---- expands ----
            # DVE: batch-0 first half, then odd batches (a 2-port-mode
            # DVE copy locks GPSIMD out of the shared SBUF port, so keep
            # DVE's queue clear while the first stores are emitted).
            Y0a = y0ap.tile([128, H0 * FL], mybir.dt.float32)
            expand(nc.vector, X0, XROW, 0, Y0a, H0 * FL, H0)
            # ACT: batch-0 second half, tail expand, then even batches
            Y0b = y0bp.tile([128, H1 * FL], mybir.dt.float32)
            expand(nc.scalar, X0, XROW, H0 * C, Y0b, H1 * FL, H1)

            # Batches 1-4: BOTH engines cooperate per batch (DVE windows
            # [0,16), ACT [16,31) into the same tile; the store waits on
            # both writers).  The clean-run ramp (10-30 us) runs at only
            # ~190-430 GB/s because expanded data isn't ready fast
            # enough; pairing makes Y1..Y4 available ~2 us earlier each.
            # The tail expand moves after the paired halves (its store
            # is reordered past Y4 to avoid head-of-line blocking).
            # Batches 5-15 alternate engines as before.
            HS = 16
            Ys = {}

            def xsrc(b):
                if b == 1:
                    return X1, XROW, 0
                if b < 8:
                    return XG1, 6 * XROW, (b - 2) * XROW
                return XG2, 8 * XROW, (b - 8) * XROW

            for b in range(1, 5):
                src_t, row, off = xsrc(b)
                Y = yp.tile([128, YROW], mybir.dt.float32)
                expand(nc.vector, src_t, row, off, Y, YROW, HS)
                expand(nc.scalar, src_t, row, off + HS * C, Y, YROW,
                       WPP - HS, HS * FL)
                Ys[b] = Y
            T = tp.tile([128, TROW], mybir.dt.float32)
            expand(nc.scalar, R, RLD, 0, T, TROW, TWIN)
            for b in range(5, B):
                src_t, row, off = xsrc(b)
                Y = yp.tile([128, YROW], mybir.dt.float32)
                eng = nc.vector if b % 2 == 1 else nc.scalar
                expand(eng, src_t, row, off, Y, YROW, WPP)
                Ys[b] = Y

            # ---- stores: ALL on GPSIMD/SWDGE ----
            # window-4064 first (no expand dependency, primes the pipe;
            # 16 partitions -> one engine, 64 KB, harmless early).
            st(nc.gpsimd, V, [[FL, 16], [1, FL]], 0,
               [[OB, B], [1, FL]], W4 * FL)
            # batch-0 halves, then the tail, then batches 1..15; the
            # queue ends on clean uniform 15.5 KB-descriptor stores.
            st(nc.gpsimd, Y0a, [[H0 * FL, 128], [1, H0 * FL]], 0,
               [[YROW, 128], [1, H0 * FL]], 0)
            st(nc.gpsimd, Y0b, [[H1 * FL, 128], [1, H1 * FL]], 0,
               [[YROW, 128], [1, H1 * FL]], H0 * FL)
            for b in range(1, 5):
                st(nc.gpsimd, Ys[b], [[YROW, 128], [1, YROW]], 0,
                   [[YROW, 128], [1, YROW]], b * OB)
            st(nc.gpsimd, T, [[TROW, 128], [1, TROW]], 0,
               [[OB, B], [TSTR * FL, TSL], [1, TROW]], NBULK * FL)
            for b in range(5, B):
                st(nc.gpsimd, Ys[b], [[YROW, 128], [1, YROW]], 0,
                   [[YROW, 128], [1, YROW]], b * OB)

    nc.finalize()
    return nc


def run_sharded(x: np.ndarray, trace: bool = False):
    """Shard batch across 8 cores, run, gather. Returns (out, raw results)."""
    if "nc" not in _cache:
        _cache["nc"] = build_nc()
    nc = _cache["nc"]

    x = np.ascontiguousarray(x, dtype=np.float32)
    in_maps = [{"x": x[i * B : (i + 1) * B]} for i in range(N_CORES)]
    res = run_bass_kernel_spmd(nc, in_maps, list(range(N_CORES)), trace=trace)
    out = np.concatenate([res.results[i]["out"] for i in range(N_CORES)], axis=0)
    return out, res


def kernel(x: np.ndarray) -> np.ndarray:
    out, _ = run_sharded(x, trace=False)
    return out

